# revision 2
# baseline (speedup 1.0000x reference)
"""Trainium2 Bass kernel for nn_BatchDotPred: per-edge dot products of
gathered node features (bf16 gathers: feat rows are down-converted to
bf16 host-side, halving SWDGE gather bytes; DVE mul in bf16, reduce to
f32 -- rel err ~2.5e-3, well inside the 2e-2 gate).

  edges: [E, 2] int, feat: [N, D] f32  ->  scores [E, 1] f32
  scores[e] = dot(feat[edges[e,0]], feat[edges[e,1]])

Strategy (8 NeuronCores, data parallel over edges):
  - E edges split into 8 contiguous shards of 250k, one per core.
  - The feat table is passed to every core as 4 chunk tensors of 25k rows
    each, so node indices local to a chunk fit the int16 index dtype of the
    InstDMAGatherAnt ucode (the fast SWDGE gather primitive; ~1.3 ns/row
    measured vs ~9 ns/row for a single SWDGE queue and far worse for generic
    indirect DMA).
  - Host-side, each core's edges are bucketed by (src_chunk, dst_chunk) -> 16
    buckets, each padded with dummy edges to a fixed capacity of CAP_TILES
    tiles x NI edges (so the single SPMD program has compile-time-constant
    shape; num_idxs_reg == NI always).
  - Per tile of NI=1024 edges: dma_gather the 1024 src rows (512 B each) from
    chunk a, dma_gather the 1024 dst rows from chunk b, DVE multiply, DVE
    segmented reduce -> [128, NI/128] scores.  Gathers cycle over the 4 SWDGE
    queues so descriptor generation runs on all four Q7 core pairs in
    parallel -- this is what reaches DMA line rate.
  - dma_gather writes row i of a tile to partition i%128, group i//128.
  - Scores accumulate in a persistent SBUF tile, written back with one DMA;
    host scatters them back to original edge order.
"""

import numpy as np

import concourse.bass as bass
import concourse.bacc as bacc
import concourse.tile as tile
import concourse.mybir as mybir
from concourse import bass_utils

BF16 = mybir.dt.np(mybir.dt.bfloat16)

N_CORES = 8
N_NODES = 100000
N_EDGES = 2000000
D = 128

N_CHUNKS = 4
CHUNK = N_NODES // N_CHUNKS          # 25000 rows per chunk tensor
N_BUCKETS = N_CHUNKS * N_CHUNKS      # 16

E_CORE = N_EDGES // N_CORES          # 250,000
BUCKET_CAP = 16384                   # edge capacity per bucket (mean 15625,
                                     # sd ~121 -> 6sd headroom)
SORT_BUCKETS = False
BUFS = 5


def set_tile_size(ni):
    """NI = edges per tile (= per dma_gather instruction)."""
    global NI, GROUPS, W, CAP_TILES, T_TOTAL, SLOTS
    NI = ni
    GROUPS = NI // 128
    W = NI // 16                     # idx columns per tile (wrapped int16)
    CAP_TILES = BUCKET_CAP // NI     # tiles per bucket
    T_TOTAL = N_BUCKETS * CAP_TILES  # tiles per core
    SLOTS = T_TOTAL * NI             # slots per core


set_tile_size(1024)

_programs = {}


def _build_program(reps=1, bufs=None):
    if bufs is None:
        bufs = BUFS
    nc = bacc.Bacc("TRN2", target_bir_lowering=False, debug=False,
                   num_devices=N_CORES, num_swdge_queues=4)
    chunk_aps = [
        nc.dram_tensor(f"feat{i}", [CHUNK, D], mybir.dt.bfloat16,
                       kind="ExternalInput").ap()
        for i in range(N_CHUNKS)
    ]
    src_ap = nc.dram_tensor("src_idx", [128, T_TOTAL * W], mybir.dt.int16,
                            kind="ExternalInput").ap()
    dst_ap = nc.dram_tensor("dst_idx", [128, T_TOTAL * W], mybir.dt.int16,
                            kind="ExternalInput").ap()
    out_ap = nc.dram_tensor("scores", [128, T_TOTAL * GROUPS],
                            mybir.dt.float32, kind="ExternalOutput").ap()

    with tile.TileContext(nc) as tc:
        with tc.tile_pool(name="pool", bufs=bufs) as pool, \
             tc.tile_pool(name="persist", bufs=1) as persist:
            src_idx = persist.tile([128, T_TOTAL * W], mybir.dt.int16)
            dst_idx = persist.tile([128, T_TOTAL * W], mybir.dt.int16)
            scores = persist.tile([128, T_TOTAL * GROUPS], mybir.dt.float32)
            nc.sync.dma_start(src_idx[:], src_ap[:])
            nc.sync.dma_start(dst_idx[:], dst_ap[:])
            q = 0
            for _ in range(reps):
                for b in range(N_BUCKETS):
                    ca, cb = b // N_CHUNKS, b % N_CHUNKS
                    for t in range(CAP_TILES):
                        gt = b * CAP_TILES + t      # global tile id
                        isl = slice(gt * W, (gt + 1) * W)
                        s_tile = pool.tile([128, NI], mybir.dt.bfloat16,
                                           tag="s")
                        d_tile = pool.tile([128, NI], mybir.dt.bfloat16,
                                           tag="d")
                        nc.gpsimd.dma_gather(
                            out_ap=s_tile[:].rearrange("p (g d) -> p g d",
                                                       d=D),
                            in_ap=chunk_aps[ca][:],
                            idxs_ap=src_idx[:, isl],
                            num_idxs=NI, num_idxs_reg=NI, elem_size=D,
                            queue_num=q % 4)
                        q += 1
                        nc.gpsimd.dma_gather(
                            out_ap=d_tile[:].rearrange("p (g d) -> p g d",
                                                       d=D),
                            in_ap=chunk_aps[cb][:],
                            idxs_ap=dst_idx[:, isl],
                            num_idxs=NI, num_idxs_reg=NI, elem_size=D,
                            queue_num=q % 4)
                        q += 1
                        nc.vector.tensor_mul(s_tile[:], s_tile[:], d_tile[:])
                        nc.vector.tensor_reduce(
                            out=scores[:, gt * GROUPS:(gt + 1) * GROUPS],
                            in_=s_tile[:].rearrange("p (g d) -> p g d", d=D),
                            axis=mybir.AxisListType.X,
                            op=mybir.AluOpType.add)
            nc.sync.dma_start(out_ap[:], scores[:])

    nc.compile()
    return nc


def _get_program(reps=1):
    key = (reps, NI, BUFS)
    if key not in _programs:
        _programs[key] = _build_program(reps)
    return _programs[key]


def _wrap_idx(idx16: np.ndarray) -> np.ndarray:
    """[T_TOTAL*NI] int16 -> [128, T_TOTAL*W] wrapped+replicated layout."""
    # per tile: [NI] -> [W, 16] -> T to [16, W]; replicate to 128 partitions
    w = idx16.reshape(T_TOTAL, W, 16).transpose(0, 2, 1)   # [T, 16, W]
    w = np.tile(w, (1, 8, 1))                              # [T, 128, W]
    return np.ascontiguousarray(
        w.transpose(1, 0, 2).reshape(128, T_TOTAL * W))


def _pack_core(src: np.ndarray, dst: np.ndarray):
    """Bucket one core's edges; returns (src_wrapped, dst_wrapped, slot2edge).

    slot2edge: [SLOTS] int64, original edge position or -1 for padding.
    Raises if any bucket overflows CAP_TILES*NI (caller falls back).
    """
    ca = src // CHUNK
    cb = dst // CHUNK
    bucket = ca * N_CHUNKS + cb
    if SORT_BUCKETS:
        # secondary sort by src row for HBM row locality in the src gathers
        order = np.lexsort((src, bucket))
    else:
        order = np.argsort(bucket, kind="stable")
    counts = np.bincount(bucket, minlength=N_BUCKETS)
    if counts.max() > CAP_TILES * NI:
        raise OverflowError(f"bucket overflow: {counts.max()}")
    starts = np.zeros(N_BUCKETS, np.int64)
    starts[1:] = np.cumsum(counts)[:-1]

    slot2edge = np.full(SLOTS, -1, np.int64)
    src_slots = np.zeros(SLOTS, np.int16)
    dst_slots = np.zeros(SLOTS, np.int16)
    # dummy edges gather row 0 of the bucket's chunks (valid local index 0)
    for b in range(N_BUCKETS):
        seg = order[starts[b]:starts[b] + counts[b]]
        base = b * CAP_TILES * NI
        slot2edge[base:base + counts[b]] = seg
        src_slots[base:base + counts[b]] = (src[seg] % CHUNK).astype(np.int16)
        dst_slots[base:base + counts[b]] = (dst[seg] % CHUNK).astype(np.int16)
    return _wrap_idx(src_slots), _wrap_idx(dst_slots), slot2edge


def _unpack_scores(out: np.ndarray, slot2edge: np.ndarray) -> np.ndarray:
    """out: [128, T_TOTAL*GROUPS] -> [E_CORE] in original edge order."""
    # slot = t*NI + g*128 + p  ->  out[p, t*GROUPS + g]
    per_slot = out.reshape(128, T_TOTAL * GROUPS).T.reshape(
        T_TOTAL, GROUPS, 128).reshape(SLOTS)
    res = np.zeros(E_CORE, np.float32)
    valid = slot2edge >= 0
    res[slot2edge[valid]] = per_slot[valid]
    return res


def _make_in_maps(edges: np.ndarray, feat: np.ndarray):
    src = np.ascontiguousarray(edges[:, 0]).astype(np.int32, copy=False)
    dst = np.ascontiguousarray(edges[:, 1]).astype(np.int32, copy=False)
    chunks = {f"feat{i}": np.ascontiguousarray(
              feat[i * CHUNK:(i + 1) * CHUNK]).astype(BF16)
              for i in range(N_CHUNKS)}
    in_maps, slot_maps = [], []
    for c in range(N_CORES):
        s = src[c * E_CORE:(c + 1) * E_CORE]
        d = dst[c * E_CORE:(c + 1) * E_CORE]
        sw, dw, s2e = _pack_core(s, d)
        in_maps.append({**chunks, "src_idx": sw, "dst_idx": dw})
        slot_maps.append(s2e)
    return in_maps, slot_maps


def _run(edges: np.ndarray, feat: np.ndarray, trace: bool = False):
    edges = np.asarray(edges)
    feat = np.ascontiguousarray(np.asarray(feat, dtype=np.float32))
    assert edges.shape == (N_EDGES, 2) and feat.shape == (N_NODES, D)
    in_maps, slot_maps = _make_in_maps(edges, feat)
    nc = _get_program()
    res = bass_utils.run_bass_kernel_spmd(
        nc, in_maps, core_ids=list(range(N_CORES)), trace=trace)
    parts = [_unpack_scores(res.results[c]["scores"], slot_maps[c])
             for c in range(N_CORES)]
    return np.concatenate(parts).astype(np.float32)[:, None], res


def kernel(edges: np.ndarray, feat: np.ndarray) -> np.ndarray:
    out, _ = _run(edges, feat, trace=False)
    return out



# revision 3
# speedup vs baseline: 1.1853x; 1.1853x over previous
"""Window-matmul Trainium2 kernel for nn_BatchDotPred (edge-major variant).

  scores[e] = dot(feat[src_e], feat[dst_e]),  E=2M, N=100k, D=128.

Per-core design (8 cores, edges sharded by dst range of 12500 nodes):
  - src rows are never gathered: edges are grouped into 128-node src windows
    (host sort); per 128-edge subtile the PE computes
    S[e, f] = sum_n onehot[n, e] * tbl_w[n, f]  (onehot host-built, bf16),
    i.e. the stationary is the subtile's one-hot and the moving tensor is the
    window's node-major table block streamed from HBM. This halves SWDGE
    descriptor load - the measured bottleneck (~2.2 ns/row over 4 queues).
  - dst rows: non-transpose SWDGE dma_gather (edge-major [128, g, 128] tiles)
    from the core's private dst chunk - int16-safe by dst-range sharding.
  - DVE multiplies S (read straight from PSUM, f32) with D into a bf16
    product tile, then reduces the feature axis into a persistent [128, NCOL]
    f32 score strip; one DMA writes all scores out at the end.
  - Window overflow (> CAPW edges) spills to 4 quadrant buckets handled by
    classic two-sided gathers (src rows addressed in the table row-major).
"""

import os

os.environ["BY_DEFAULT_DISABLE_SUBTILE_DEPS"] = "1"

import numpy as np

import concourse.bass as bass
import concourse.bacc as bacc
import concourse.tile as tile
import concourse.mybir as mybir
from concourse import bass_utils

BF16 = mybir.dt.np(mybir.dt.bfloat16)
FP8 = mybir.dt.np(mybir.dt.float8e4)

N_CORES = 8
N_NODES = 100000
N_EDGES = 2000000
D = 128

DCHUNK = N_NODES // N_CORES          # 12500 dst nodes per core
DPAD = 12544
WSZ = 128                            # src window size
NW = (N_NODES + WSZ - 1) // WSZ      # 782
WPS = 16                             # windows per strip
NSTRIP = -(-NW // WPS)               # 49
NWP = NSTRIP * WPS                   # 784
CAPW = 384                           # slots per window (3 subtiles of 128)
STRIP = WPS * CAPW                   # 6144 slots (= 6 gathers of 1024)
SLOTS_MAIN = NSTRIP * STRIP          # 301056
NSQ = 4
SPC = 512
SPILL = NSQ * SPC                    # 2048
SLOTS = SLOTS_MAIN + SPILL           # 303104
NI = 1024                            # dst gather tile (non-transpose)
NSUB = STRIP // 128                  # 48 subtiles per strip
NCOL = SLOTS // 128                  # 2368 score columns

_programs = {}


def _build_program():
    nc = bacc.Bacc("TRN2", target_bir_lowering=False, debug=False,
                   num_devices=N_CORES, num_swdge_queues=4)
    f32, bf16, i16 = mybir.dt.float32, mybir.dt.bfloat16, mybir.dt.int16
    tbl_ap = nc.dram_tensor("tblnm", [128, NWP * 128], bf16,
                            kind="ExternalInput").ap()
    dch_ap = nc.dram_tensor("dchunk", [DPAD, D], bf16,
                            kind="ExternalInput").ap()
    oh_ap = nc.dram_tensor("onehot", [128, NWP * CAPW],
                           mybir.dt.float8e4,
                           kind="ExternalInput").ap()
    didx_ap = nc.dram_tensor("dstidx", [128, SLOTS_MAIN // 16], i16,
                             kind="ExternalInput").ap()
    spsi_ap = nc.dram_tensor("spsrcidx", [128, SPILL // 16], i16,
                             kind="ExternalInput").ap()
    spdi_ap = nc.dram_tensor("spdstidx", [128, SPILL // 16], i16,
                             kind="ExternalInput").ap()
    out_ap = nc.dram_tensor("scores", [128, NCOL], f32,
                            kind="ExternalOutput").ap()
    # table rows viewed row-major for spill src gathers: row r = p*NWP + w
    srcq = [tbl_ap[32 * q:32 * (q + 1), :].rearrange("p (w f) -> (p w) f", f=D)
            for q in range(NSQ)]

    with tile.TileContext(nc) as tc:
        with tc.tile_pool(name="strips", bufs=2) as strips, \
             tc.tile_pool(name="small", bufs=3) as small, \
             tc.tile_pool(name="single", bufs=1) as single, \
             tc.tile_pool(name="ps1", bufs=3, space="PSUM") as psum1:
            scores = single.tile([128, NCOL], f32)
            qq = [0]

            def issue_gathers(st):
                didx = small.tile([128, STRIP // 16], i16, tag="didx",
                                  bufs=4, name=f"didx{st}")
                nc.sync.dma_start(
                    didx[:],
                    didx_ap[:, st * (STRIP // 16):(st + 1) * (STRIP // 16)])
                dts = []
                for g in range(STRIP // NI):
                    dt = strips.tile([128, NI], bf16, tag=f"d{g}", bufs=4,
                                     name=f"dt{st}_{g}")
                    nc.gpsimd.dma_gather(
                        out_ap=dt[:].rearrange("p (g d) -> p g d", d=D),
                        in_ap=dch_ap[:],
                        idxs_ap=didx[:, g * (NI // 16):(g + 1) * (NI // 16)],
                        num_idxs=NI, num_idxs_reg=NI, elem_size=D,
                        queue_num=qq[0] % 4)
                    qq[0] += 1
                    dts.append(dt)
                return dts

            def process(st, dts):
                tbs = small.tile([128, WPS * 128], bf16, tag="tbs",
                                 name=f"tbs{st}")
                nc.sync.dma_start(
                    tbs[:], tbl_ap[:, st * WPS * 128:(st + 1) * WPS * 128])
                ohs = small.tile([128, WPS * CAPW], mybir.dt.float8e4,
                                 tag="ohs", name=f"ohs{st}")
                nc.sync.dma_start(
                    ohs[:], oh_ap[:, st * WPS * CAPW:(st + 1) * WPS * CAPW])
                for bank in range(NSUB // 4):
                    ps1t = psum1.tile([128, 512], f32, tag="ps1",
                                      name=f"ps1_{st}_{bank}")
                    for k4 in range(4):
                        sub = bank * 4 + k4
                        j, k = divmod(sub, 3)
                        nc.tensor.matmul(
                            ps1t[:, k4 * 128:(k4 + 1) * 128],
                            ohs[:, j * CAPW + k * 128:j * CAPW + (k + 1) * 128],
                            tbs[:, j * 128:(j + 1) * 128])
                    prod = strips.tile([128, 512], bf16, tag="prod", bufs=3,
                                       name=f"prod_{st}_{bank}")
                    gt, h = divmod(bank, 2)
                    nc.vector.tensor_mul(prod[:], ps1t[:],
                                         dts[gt][:, h * 512:(h + 1) * 512])
                    nc.vector.tensor_reduce(
                        out=scores[:, st * NSUB + bank * 4:
                                   st * NSUB + bank * 4 + 4],
                        in_=prod[:].rearrange("p (g d) -> p g d", d=D),
                        axis=mybir.AxisListType.X,
                        op=mybir.AluOpType.add)

            pend = []
            for st in range(NSTRIP):
                pend.append((st, issue_gathers(st)))
                if len(pend) > 2:
                    process(*pend.pop(0))
            for item in pend:
                process(*item)

            # ---- spill path ----
            spdi = single.tile([128, SPILL // 16], i16)
            nc.sync.dma_start(spdi[:], spdi_ap[:])
            spsi = single.tile([128, SPILL // 16], i16)
            nc.sync.dma_start(spsi[:], spsi_ap[:])
            spd = single.tile([128, SPILL], bf16)
            for g in range(SPILL // NI):
                nc.gpsimd.dma_gather(
                    out_ap=spd[:, g * NI:(g + 1) * NI].rearrange(
                        "p (g d) -> p g d", d=D),
                    in_ap=dch_ap[:],
                    idxs_ap=spdi[:, g * (NI // 16):(g + 1) * (NI // 16)],
                    num_idxs=NI, num_idxs_reg=NI, elem_size=D,
                    queue_num=qq[0] % 4)
                qq[0] += 1
            sps = single.tile([128, SPILL], bf16)
            for sq in range(NSQ):
                nc.gpsimd.dma_gather(
                    out_ap=sps[:, sq * SPC:(sq + 1) * SPC].rearrange(
                        "p (g d) -> p g d", d=D),
                    in_ap=srcq[sq],
                    idxs_ap=spsi[:, sq * (SPC // 16):(sq + 1) * (SPC // 16)],
                    num_idxs=SPC, num_idxs_reg=SPC, elem_size=D,
                    queue_num=qq[0] % 4)
                qq[0] += 1
            spp = single.tile([128, SPILL], bf16)
            nc.vector.tensor_mul(spp[:], sps[:], spd[:])
            nc.vector.tensor_reduce(
                out=scores[:, NSTRIP * NSUB:NSTRIP * NSUB + SPILL // 128],
                in_=spp[:].rearrange("p (g d) -> p g d", d=D),
                axis=mybir.AxisListType.X,
                op=mybir.AluOpType.add)
            nc.sync.dma_start(out_ap[:], scores[:])

    nc.compile()
    return nc


def _get_program():
    if "p" not in _programs:
        _programs["p"] = _build_program()
    return _programs["p"]


def _wrap_idx(idx16: np.ndarray, ni: int) -> np.ndarray:
    """[G*ni] int16 -> [128, G*(ni//16)] wrapped + replicated layout."""
    g = idx16.size // ni
    w = ni // 16
    a = idx16.reshape(g, w, 16).transpose(0, 2, 1)
    a = np.tile(a, (1, 8, 1))
    return np.ascontiguousarray(a.transpose(1, 0, 2).reshape(128, g * w))


def _pack_core(src, dst, eids, base):
    """One core's edges (dst in [base, base+DCHUNK))."""
    w = src >> 7
    order = np.argsort(w, kind="stable")
    ws = w[order]
    counts = np.bincount(ws, minlength=NW)
    starts = np.zeros(NW, np.int64)
    starts[1:] = np.cumsum(counts)[:-1]

    slot2edge = np.full(SLOTS, -1, np.int64)
    dst_local = np.zeros(SLOTS_MAIN, np.int16)
    spill = []
    for wi in np.nonzero(counts)[0]:
        k = counts[wi]
        s0 = starts[wi]
        take = min(k, CAPW)
        stp, j = divmod(wi, WPS)
        b = stp * STRIP + j * CAPW
        sel = order[s0:s0 + take]
        slot2edge[b:b + take] = sel
        dst_local[b:b + take] = (dst[sel] - base).astype(np.int16)
        if k > take:
            spill.append(order[s0 + take:s0 + k])

    oh = np.zeros((128, NWP * CAPW), FP8)
    sidx = np.nonzero(slot2edge[:SLOTS_MAIN] >= 0)[0]
    stp = sidx // STRIP
    r = sidx % STRIP
    col = (stp * WPS + r // CAPW) * CAPW + r % CAPW
    e = slot2edge[sidx]
    oh[src[e] & 127, col] = 1.0

    sp_dst = np.zeros(SPILL, np.int16)
    sp_src = np.zeros(SPILL, np.int16)
    if spill:
        spill = np.concatenate(spill)
        quad = (src[spill] & 127) >> 5
        for sq in range(NSQ):
            lst = spill[quad == sq]
            if lst.size > SPC:
                raise OverflowError(f"spill bucket {sq}: {lst.size}")
            b = sq * SPC
            slot2edge[SLOTS_MAIN + b:SLOTS_MAIN + b + lst.size] = lst
            sp_dst[b:b + lst.size] = (dst[lst] - base).astype(np.int16)
            sp_src[b:b + lst.size] = (
                ((src[lst] & 127) - 32 * sq) * NWP + (src[lst] >> 7)
            ).astype(np.int16)

    return {
        "onehot": oh,
        "dstidx": _wrap_idx(dst_local, NI),
        "spdstidx": _wrap_idx(sp_dst, NI),
        "spsrcidx": _wrap_idx(sp_src, SPC),
    }, slot2edge, eids


def _prep_shared(feat):
    feat_bf = feat.astype(BF16)
    t = np.zeros((NWP * 128, D), BF16)
    t[:N_NODES] = feat_bf
    tblnm = np.ascontiguousarray(
        t.reshape(NWP, 128, D).transpose(1, 0, 2).reshape(128, NWP * 128))
    chunks = []
    for c in range(N_CORES):
        ch = np.zeros((DPAD, D), BF16)
        ch[:DCHUNK] = feat_bf[c * DCHUNK:(c + 1) * DCHUNK]
        chunks.append(ch)
    return tblnm, chunks


def _run(edges: np.ndarray, feat: np.ndarray, trace: bool = False):
    edges = np.asarray(edges)
    feat = np.ascontiguousarray(np.asarray(feat, dtype=np.float32))
    assert edges.shape == (N_EDGES, 2) and feat.shape == (N_NODES, D)
    src = edges[:, 0].astype(np.int64)
    dst = edges[:, 1].astype(np.int64)
    tblnm, chunks = _prep_shared(feat)

    core_of = dst // DCHUNK
    in_maps, slot_maps, eid_maps = [], [], []
    for c in range(N_CORES):
        sel = np.nonzero(core_of == c)[0]
        m, s2e, eids = _pack_core(src[sel], dst[sel], sel, c * DCHUNK)
        m["tblnm"] = tblnm
        m["dchunk"] = chunks[c]
        in_maps.append(m)
        slot_maps.append(s2e)
        eid_maps.append(eids)

    nc = _get_program()
    res = bass_utils.run_bass_kernel_spmd(
        nc, in_maps, core_ids=list(range(N_CORES)), trace=trace)

    out = np.zeros(N_EDGES, np.float32)
    for c in range(N_CORES):
        # slot s -> scores[s % 128, s // 128]
        flat = res.results[c]["scores"].T.reshape(SLOTS)
        s2e = slot_maps[c]
        valid = s2e >= 0
        out[eid_maps[c][s2e[valid]]] = flat[valid]
    return out[:, None], res


def kernel(edges: np.ndarray, feat: np.ndarray) -> np.ndarray:
    out, _ = _run(edges, feat, trace=False)
    return out


# revision 4
# speedup vs baseline: 1.3995x; 1.1808x over previous
"""Window-matmul Trainium2 kernel for nn_BatchDotPred (edge-major variant).

  scores[e] = dot(feat[src_e], feat[dst_e]),  E=2M, N=100k, D=128.

Per-core design (8 cores, edges sharded by dst range of 12500 nodes):
  - src rows are never gathered: edges are grouped into 128-node src windows
    (host sort); per 128-edge subtile the PE computes
    S[e, f] = sum_n onehot[n, e] * tbl_w[n, f]  (onehot host-built, bf16),
    i.e. the stationary is the subtile's one-hot and the moving tensor is the
    window's node-major table block streamed from HBM. This halves SWDGE
    descriptor load - the measured bottleneck (~2.2 ns/row over 4 queues).
  - dst rows: non-transpose SWDGE dma_gather (edge-major [128, g, 128] tiles)
    from the core's private dst chunk - int16-safe by dst-range sharding.
  - DVE multiplies S (read straight from PSUM, f32) with D into a bf16
    product tile, then reduces the feature axis into a persistent [128, NCOL]
    f32 score strip; one DMA writes all scores out at the end.
  - Window overflow (> CAPW edges) spills to 4 quadrant buckets handled by
    classic two-sided gathers (src rows addressed in the table row-major).
"""

import os

os.environ["BY_DEFAULT_DISABLE_SUBTILE_DEPS"] = "1"

import numpy as np

import concourse.bass as bass
import concourse.bacc as bacc
import concourse.tile as tile
import concourse.mybir as mybir
from concourse import bass_utils

BF16 = mybir.dt.np(mybir.dt.bfloat16)
FP8 = mybir.dt.np(mybir.dt.float8e4)

N_CORES = 8
N_NODES = 100000
N_EDGES = 2000000
D = 128

DCHUNK = N_NODES // N_CORES          # 12500 dst nodes per core
DPAD = 12544
WSZ = 96                             # src window size
NW = (N_NODES + WSZ - 1) // WSZ      # 1042
WPS = 16                             # windows per strip
NSTRIP = -(-NW // WPS)               # 66
NWP = NSTRIP * WPS                   # 1056
CAPW = 256                           # slots per window (2 subtiles of 128)
STRIP = WPS * CAPW                   # 4096 slots (= 4 gathers of 1024)
SLOTS_MAIN = NSTRIP * STRIP          # 301056
NSQ = 4
SPC = 512
SPILL = NSQ * SPC                    # 2048
SLOTS = SLOTS_MAIN + SPILL           # 303104
NI = 1024                            # dst gather tile (non-transpose)
NSUB = STRIP // 128                  # 48 subtiles per strip
NCOL = SLOTS // 128                  # 2368 score columns

_programs = {}


def _build_program():
    nc = bacc.Bacc("TRN2", target_bir_lowering=False, debug=False,
                   num_devices=N_CORES, num_swdge_queues=4)
    f32, bf16, i16 = mybir.dt.float32, mybir.dt.bfloat16, mybir.dt.int16
    tbl_ap = nc.dram_tensor("tblnm", [128, NWP * 128], bf16,
                            kind="ExternalInput").ap()
    dch_ap = nc.dram_tensor("dchunk", [DPAD, D], bf16,
                            kind="ExternalInput").ap()
    oh_ap = nc.dram_tensor("onehot", [128, NWP * CAPW],
                           mybir.dt.float8e4,
                           kind="ExternalInput").ap()
    didx_ap = nc.dram_tensor("dstidx", [128, SLOTS_MAIN // 16], i16,
                             kind="ExternalInput").ap()
    spsi_ap = nc.dram_tensor("spsrcidx", [128, SPILL // 16], i16,
                             kind="ExternalInput").ap()
    spdi_ap = nc.dram_tensor("spdstidx", [128, SPILL // 16], i16,
                             kind="ExternalInput").ap()
    out_ap = nc.dram_tensor("scores", [128, NCOL], f32,
                            kind="ExternalOutput").ap()
    # table rows viewed row-major for spill src gathers: row r = p*NWP + w
    srcq = [tbl_ap[24 * q:24 * (q + 1), :].rearrange("p (w f) -> (p w) f", f=D)
            for q in range(NSQ)]

    with tile.TileContext(nc) as tc:
        with tc.tile_pool(name="strips", bufs=2) as strips, \
             tc.tile_pool(name="small", bufs=3) as small, \
             tc.tile_pool(name="single", bufs=1) as single, \
             tc.tile_pool(name="ps1", bufs=3, space="PSUM") as psum1:
            scores = single.tile([128, NCOL], f32)
            qq = [0]

            def issue_gathers(st):
                didx = small.tile([128, STRIP // 16], i16, tag="didx",
                                  bufs=4, name=f"didx{st}")
                nc.sync.dma_start(
                    didx[:],
                    didx_ap[:, st * (STRIP // 16):(st + 1) * (STRIP // 16)])
                dts = []
                for g in range(STRIP // NI):
                    dt = strips.tile([128, NI], bf16, tag=f"d{g}", bufs=4,
                                     name=f"dt{st}_{g}")
                    nc.gpsimd.dma_gather(
                        out_ap=dt[:].rearrange("p (g d) -> p g d", d=D),
                        in_ap=dch_ap[:],
                        idxs_ap=didx[:, g * (NI // 16):(g + 1) * (NI // 16)],
                        num_idxs=NI, num_idxs_reg=NI, elem_size=D,
                        queue_num=qq[0] % 4)
                    qq[0] += 1
                    dts.append(dt)
                return dts

            def process(st, dts):
                tbs = small.tile([128, WPS * 128], bf16, tag="tbs",
                                 name=f"tbs{st}")
                nc.sync.dma_start(
                    tbs[:], tbl_ap[:, st * WPS * 128:(st + 1) * WPS * 128])
                ohs = small.tile([128, WPS * CAPW], mybir.dt.float8e4,
                                 tag="ohs", name=f"ohs{st}")
                nc.sync.dma_start(
                    ohs[:], oh_ap[:, st * WPS * CAPW:(st + 1) * WPS * CAPW])
                for bank in range(NSUB // 4):
                    ps1t = psum1.tile([128, 512], f32, tag="ps1",
                                      name=f"ps1_{st}_{bank}")
                    for k4 in range(4):
                        sub = bank * 4 + k4
                        j, k = divmod(sub, 2)
                        nc.tensor.matmul(
                            ps1t[:, k4 * 128:(k4 + 1) * 128],
                            ohs[0:WSZ, j * CAPW + k * 128:
                                j * CAPW + (k + 1) * 128],
                            tbs[0:WSZ, j * 128:(j + 1) * 128])
                    prod = strips.tile([128, 512], bf16, tag="prod", bufs=3,
                                       name=f"prod_{st}_{bank}")
                    gt, h = divmod(bank, 2)
                    nc.vector.tensor_mul(prod[:], ps1t[:],
                                         dts[gt][:, h * 512:(h + 1) * 512])
                    nc.vector.tensor_reduce(
                        out=scores[:, st * NSUB + bank * 4:
                                   st * NSUB + bank * 4 + 4],
                        in_=prod[:].rearrange("p (g d) -> p g d", d=D),
                        axis=mybir.AxisListType.X,
                        op=mybir.AluOpType.add)

            pend = []
            for st in range(NSTRIP):
                pend.append((st, issue_gathers(st)))
                if len(pend) > 2:
                    process(*pend.pop(0))
            for item in pend:
                process(*item)

            # ---- spill path ----
            spdi = single.tile([128, SPILL // 16], i16)
            nc.sync.dma_start(spdi[:], spdi_ap[:])
            spsi = single.tile([128, SPILL // 16], i16)
            nc.sync.dma_start(spsi[:], spsi_ap[:])
            spd = single.tile([128, SPILL], bf16)
            for g in range(SPILL // NI):
                nc.gpsimd.dma_gather(
                    out_ap=spd[:, g * NI:(g + 1) * NI].rearrange(
                        "p (g d) -> p g d", d=D),
                    in_ap=dch_ap[:],
                    idxs_ap=spdi[:, g * (NI // 16):(g + 1) * (NI // 16)],
                    num_idxs=NI, num_idxs_reg=NI, elem_size=D,
                    queue_num=qq[0] % 4)
                qq[0] += 1
            sps = single.tile([128, SPILL], bf16)
            for sq in range(NSQ):
                nc.gpsimd.dma_gather(
                    out_ap=sps[:, sq * SPC:(sq + 1) * SPC].rearrange(
                        "p (g d) -> p g d", d=D),
                    in_ap=srcq[sq],
                    idxs_ap=spsi[:, sq * (SPC // 16):(sq + 1) * (SPC // 16)],
                    num_idxs=SPC, num_idxs_reg=SPC, elem_size=D,
                    queue_num=qq[0] % 4)
                qq[0] += 1
            spp = single.tile([128, SPILL], bf16)
            nc.vector.tensor_mul(spp[:], sps[:], spd[:])
            nc.vector.tensor_reduce(
                out=scores[:, NSTRIP * NSUB:NSTRIP * NSUB + SPILL // 128],
                in_=spp[:].rearrange("p (g d) -> p g d", d=D),
                axis=mybir.AxisListType.X,
                op=mybir.AluOpType.add)
            nc.sync.dma_start(out_ap[:], scores[:])

    nc.compile()
    return nc


def _get_program():
    if "p" not in _programs:
        _programs["p"] = _build_program()
    return _programs["p"]


def _wrap_idx(idx16: np.ndarray, ni: int) -> np.ndarray:
    """[G*ni] int16 -> [128, G*(ni//16)] wrapped + replicated layout."""
    g = idx16.size // ni
    w = ni // 16
    a = idx16.reshape(g, w, 16).transpose(0, 2, 1)
    a = np.tile(a, (1, 8, 1))
    return np.ascontiguousarray(a.transpose(1, 0, 2).reshape(128, g * w))


def _pack_core(src, dst, eids, base):
    """One core's edges (dst in [base, base+DCHUNK))."""
    w = src // WSZ
    order = np.argsort(w, kind="stable")
    ws = w[order]
    counts = np.bincount(ws, minlength=NW)
    starts = np.zeros(NW, np.int64)
    starts[1:] = np.cumsum(counts)[:-1]

    slot2edge = np.full(SLOTS, -1, np.int64)
    dst_local = np.zeros(SLOTS_MAIN, np.int16)
    spill = []
    for wi in np.nonzero(counts)[0]:
        k = counts[wi]
        s0 = starts[wi]
        take = min(k, CAPW)
        stp, j = divmod(wi, WPS)
        b = stp * STRIP + j * CAPW
        sel = order[s0:s0 + take]
        slot2edge[b:b + take] = sel
        dst_local[b:b + take] = (dst[sel] - base).astype(np.int16)
        if k > take:
            spill.append(order[s0 + take:s0 + k])

    oh = np.zeros((128, NWP * CAPW), FP8)
    sidx = np.nonzero(slot2edge[:SLOTS_MAIN] >= 0)[0]
    stp = sidx // STRIP
    r = sidx % STRIP
    col = (stp * WPS + r // CAPW) * CAPW + r % CAPW
    e = slot2edge[sidx]
    oh[src[e] % WSZ, col] = 1.0

    sp_dst = np.zeros(SPILL, np.int16)
    sp_src = np.zeros(SPILL, np.int16)
    if spill:
        spill = np.concatenate(spill)
        quad = (src[spill] % WSZ) // 24
        for sq in range(NSQ):
            lst = spill[quad == sq]
            if lst.size > SPC:
                raise OverflowError(f"spill bucket {sq}: {lst.size}")
            b = sq * SPC
            slot2edge[SLOTS_MAIN + b:SLOTS_MAIN + b + lst.size] = lst
            sp_dst[b:b + lst.size] = (dst[lst] - base).astype(np.int16)
            sp_src[b:b + lst.size] = (
                ((src[lst] % WSZ) - 24 * sq) * NWP + (src[lst] // WSZ)
            ).astype(np.int16)

    return {
        "onehot": oh,
        "dstidx": _wrap_idx(dst_local, NI),
        "spdstidx": _wrap_idx(sp_dst, NI),
        "spsrcidx": _wrap_idx(sp_src, SPC),
    }, slot2edge, eids


def _prep_shared(feat):
    feat_bf = feat.astype(BF16)
    t = np.zeros((NWP * WSZ, D), BF16)
    t[:N_NODES] = feat_bf
    t = t.reshape(NWP, WSZ, D)
    full = np.zeros((NWP, 128, D), BF16)
    full[:, :WSZ] = t
    tblnm = np.ascontiguousarray(
        full.transpose(1, 0, 2).reshape(128, NWP * 128))
    chunks = []
    for c in range(N_CORES):
        ch = np.zeros((DPAD, D), BF16)
        ch[:DCHUNK] = feat_bf[c * DCHUNK:(c + 1) * DCHUNK]
        chunks.append(ch)
    return tblnm, chunks


def _run(edges: np.ndarray, feat: np.ndarray, trace: bool = False):
    edges = np.asarray(edges)
    feat = np.ascontiguousarray(np.asarray(feat, dtype=np.float32))
    assert edges.shape == (N_EDGES, 2) and feat.shape == (N_NODES, D)
    src = edges[:, 0].astype(np.int64)
    dst = edges[:, 1].astype(np.int64)
    tblnm, chunks = _prep_shared(feat)

    core_of = dst // DCHUNK
    in_maps, slot_maps, eid_maps = [], [], []
    for c in range(N_CORES):
        sel = np.nonzero(core_of == c)[0]
        m, s2e, eids = _pack_core(src[sel], dst[sel], sel, c * DCHUNK)
        m["tblnm"] = tblnm
        m["dchunk"] = chunks[c]
        in_maps.append(m)
        slot_maps.append(s2e)
        eid_maps.append(eids)

    nc = _get_program()
    res = bass_utils.run_bass_kernel_spmd(
        nc, in_maps, core_ids=list(range(N_CORES)), trace=trace)

    out = np.zeros(N_EDGES, np.float32)
    for c in range(N_CORES):
        # slot s -> scores[s % 128, s // 128]
        flat = res.results[c]["scores"].T.reshape(SLOTS)
        s2e = slot_maps[c]
        valid = s2e >= 0
        out[eid_maps[c][s2e[valid]]] = flat[valid]
    return out[:, None], res


def kernel(edges: np.ndarray, feat: np.ndarray) -> np.ndarray:
    out, _ = _run(edges, feat, trace=False)
    return out
